# revision 13
# baseline (speedup 1.0000x reference)
"""BiDiTreeLSTM Trainium2 kernel.

Full-input contract: kernel(**inputs) takes the unsharded numpy inputs of
reference.setup_inputs() and returns the full [64, 512] output.

Strategy: data-parallel over trees (8 trees per NeuronCore, 8 cores).
Per-core layout is feature-major: node-state tensors live in SBUF as
[128 partitions, 2 feature halves, cols], cols level-major (tree-major
within a level); children of parent column c are columns 2c/2c+1 of the
next level.

Precision plan (validated vs the jax reference in numpy sim; combined rel
err ~6e-3 against the 2e-2 gate):
 - X-projections (W_iou_bu@X, Wx_td@X) and U_iou_bu@hsum: bf16 operands
   (X plain, weights host-scaled x256).  bf16 matmul = 1 cycle/row at any
   moving dim (fp32r needs >=256), FWL halves LDWEIGHTS, X DMA halves.
 - Wh_td@h_bu, U_iou_td@h_td, U_f_{bu,td}@h: fp8e4 DoubleRow (one matmul
   per 256-contraction instead of two; 0.5 cycles/row).  Weights host-
   scaled x256; h shadows are UNSCALED plain fp8 casts (sim: scaled vs
   unscaled identical).  fp8 on U_iou_bu would fail the accuracy gate
   (2e-2) so that path stays bf16.
 - Every gate/f psum is x256; 1/256 is folded into the activation scale.
h_bu is bf16 (feeds bf16 hsum + f32-staged root output) plus an fp8 shadow
for DoubleRow consumers; TD h state is fp8-only.  fp8 DoubleRow APs keep
the feature-pair dim stride a multiple of 16 bytes.

Zero-fills exploited per the problem spec: h0 == c0 == 0, all biases == 0.
"""

import numpy as np
import ml_dtypes

B, NN, XS, H = 64, 1023, 256, 256
NCORES = 8
DEPTH = 9  # levels 0..9, level l has 2^l nodes per tree
TMAX = 512
SM_LEV = 6  # levels 0..SM_LEV-1 get batched W-projections (pre_bu / pre_td)
TOTP = 8192  # hbu8 padded col count (stride % 16 == 0)

E4 = ml_dtypes.float8_e4m3fn
BF16 = ml_dtypes.bfloat16

_CACHE = {}

LAST_EXEC_NS = None


def _levels(bl):
    levw = [bl * (1 << l) for l in range(DEPTH + 1)]
    levo = [bl * ((1 << l) - 1) for l in range(DEPTH + 1)]
    tot = bl * NN
    return levw, levo, tot


def _build_nc(bl):
    from concourse import bacc
    import concourse.mybir as mybir
    import concourse.tile as tile

    f32 = mybir.dt.float32
    bf16 = mybir.dt.bfloat16
    f8 = mybir.dt.float8e4
    DR = mybir.MatmulPerfMode.DoubleRow
    Sig = mybir.ActivationFunctionType.Sigmoid
    Tanh = mybir.ActivationFunctionType.Tanh
    DS = 1.0 / 256.0  # descale folded into the psum-reading activations

    LEVW, LEVO, TOT = _levels(bl)
    SM = LEVO[SM_LEV]  # cols of levels 0..SM_LEV-1 (contiguous, level-major)
    SMP = 512  # padded stride for the xsm tile

    nc = bacc.Bacc("TRN2", target_bir_lowering=False)

    x16_d = nc.declare_dram_parameter("x16", [256, TOT], bf16, isOutput=False)
    w16_d = nc.declare_dram_parameter("w_bu16", [256, 3 * H], bf16, isOutput=False)
    ub_d = nc.declare_dram_parameter("u_bu16", [256, 3 * H], bf16, isOutput=False)
    wx16_d = nc.declare_dram_parameter("wx_td16", [256, 3 * H], bf16, isOutput=False)
    wh8_d = nc.declare_dram_parameter("wh_td8", [256, 3 * H], f8, isOutput=False)
    ut8_d = nc.declare_dram_parameter("u_td8", [256, 3 * H], f8, isOutput=False)
    ufb8_d = nc.declare_dram_parameter("uf_bu8", [256, H], f8, isOutput=False)
    uft8_d = nc.declare_dram_parameter("uf_td8", [256, H], f8, isOutput=False)
    out_d = nc.declare_dram_parameter("out", [512, bl], f32, isOutput=True)

    with tile.TileContext(nc) as tc:
        with (
            tc.tile_pool(name="const", bufs=1) as const,
            tc.tile_pool(name="hbu_pool", bufs=1) as hbu_pool,
            tc.tile_pool(name="work", bufs=2) as work,
            tc.tile_pool(name="xtp", bufs=3) as xtp,
            tc.tile_pool(name="psg", bufs=1, space="PSUM") as psg,
            tc.tile_pool(name="psf", bufs=1, space="PSUM") as psf,
        ):
            def load_16(dram, cols, nm, eng):
                t = const.tile([128, 2, cols], bf16, name=nm, tag=nm)
                for k in (0, 1):
                    eng.dma_start(out=t[:, k, :], in_=dram[k * 128:(k + 1) * 128, :])
                return t

            def load_8(dram, cols, nm, eng):
                t = const.tile([128, 2, cols], f8, name=nm, tag=nm)
                for k in (0, 1):
                    eng.dma_start(out=t[:, k, :], in_=dram[k * 128:(k + 1) * 128, :])
                return t

            # bu weights first: w16 on the sync queue ahead of the xt
            # stream (mm#0 needs it); ub16/ufb8 on scalar (acts start later)
            w16 = load_16(w16_d, 3 * H, "w16", nc.scalar)
            ub16 = load_16(ub_d, 3 * H, "ub16", nc.scalar)
            ufb8 = load_8(ufb8_d, H, "ufb8", nc.scalar)
            # td weights are DMA'd later (emitted after the BU leaf level)
            wx16 = const.tile([128, 2, 3 * H], bf16, name="wx16", tag="wx16")
            wh8 = const.tile([128, 2, 3 * H], f8, name="wh8", tag="wh8")
            ut8 = const.tile([128, 2, 3 * H], f8, name="ut8", tag="ut8")
            uft8 = const.tile([128, 2, H], f8, name="uft8", tag="uft8")

            def load_td_weights():
                # sync queue is idle once the last BU xt tiles are in flight
                for k in (0, 1):
                    nc.sync.dma_start(out=wx16[:, k, :], in_=wx16_d[k * 128:(k + 1) * 128, :])
                    nc.sync.dma_start(out=wh8[:, k, :], in_=wh8_d[k * 128:(k + 1) * 128, :])
                    nc.sync.dma_start(out=ut8[:, k, :], in_=ut8_d[k * 128:(k + 1) * 128, :])
                    nc.sync.dma_start(out=uft8[:, k, :], in_=uft8_d[k * 128:(k + 1) * 128, :])

            # h_bu: bf16 primary (hsum, root out) + fp8 shadow (DR rhs)
            hbu = hbu_pool.tile([128, 2 * TOT], bf16, name="hbu", tag="hbu")
            hbu8 = hbu_pool.tile([128, 2, TOTP], f8, name="hbu8", tag="hbu8")
            mean = const.tile([128, 2, bl], f32, name="mean", tag="mean")
            root_f32 = const.tile([128, 2, bl], f32, name="rootf", tag="rootf")

            xsm = const.tile([128, 2, SMP], bf16, name="xsm", tag="xsm")

            # HAM-warming filler: dead matmuls on a scratch tile keep the PE
            # activity window busy through DMA startup and the tiny levels,
            # so real matmuls run at 2.4 GHz instead of the cold 1.2.
            junk = const.tile([128, 2, 512], bf16, name="junk", tag="junk")
            nc.gpsimd.memset(junk, 0.0)

            def filler(n, cols=512):
                for _ in range(n):
                    pj = psf.tile([128, cols], f32, name="junk_ps", tag="pf")
                    nc.tensor.matmul(
                        pj, junk[:, 0, 0:128], junk[:, 1, 0:cols],
                        start=True, stop=True,
                    )

            def load_xsm():
                for k in (0, 1):
                    nc.sync.dma_start(out=xsm[:, k, 0:SM], in_=x16_d[k * 128:(k + 1) * 128, 0:SM])

            def load_x(off, o0, T):
                xt = xtp.tile([128, 2, T], bf16, name="xt", tag="xt", bufs=3)
                for k in (0, 1):
                    nc.sync.dma_start(out=xt[:, k, :], in_=x16_d[k * 128:(k + 1) * 128, off + o0:off + o0 + T])
                return xt

            def g2(ap, width):
                return ap.rearrange("p (g c) -> p g c", g=2)

            def wsl(wt, q, g):
                m = 2 * q + g
                return wt[:, :, m * 128:(m + 1) * 128]

            # ---- per-tile gate matmuls -------------------------------------
            # Three per-gate psum tiles [128, 2T] (tags pgi/pgo/pgu, bufs=1):
            # the next tile's gate-q matmuls wait only on this tile's gate-q
            # activation.  phase1 = input-ready matmuls, phase2 = recurrence-
            # dependent ones, deferred (groups left open) when each T slice
            # is a whole PSUM bank (T == 512), else merged.
            def iou_mms(T, phase1, phase2=None):
                """Small levels (T <= 256) share one [128, 4T] i|o psum tile
                (tag pgi, same banks) so a single fused Sigmoid evacuates
                both gates; big tiles keep three per-gate tiles for bank-
                level pipelining with deferred phase2."""
                fused = T <= 256
                merged = phase2 is not None and (T < 512 or fused)
                p1 = phase1 + phase2 if merged else phase1
                pending = phase2 is not None and not merged
                if fused:
                    pio = psg.tile([128, 4 * T], f32, name="pio", tag="pgi")
                    pu = psg.tile([128, 2 * T], f32, name="pu", tag="pgu")
                    pg = (pio, pu)

                    def slot(q, g):
                        if q < 2:
                            return pio[:, (2 * q + g) * T:(2 * q + g + 1) * T]
                        return pu[:, g * T:(g + 1) * T]
                else:
                    tiles = []
                    for tag in ("pgi", "pgo", "pgu"):
                        tiles.append(psg.tile([128, 2 * T], f32, name=tag, tag=tag))
                    pg = tuple(tiles)

                    def slot(q, g):
                        return pg[q][:, g * T:(g + 1) * T]

                def emit(q, g, mms, start, stop):
                    for i, (lhs, rhs, pm) in enumerate(mms):
                        nc.tensor.matmul(
                            slot(q, g), lhs, rhs,
                            start=(start and i == 0),
                            stop=(stop and i == len(mms) - 1),
                            perf_mode=pm,
                        )

                for q in range(3):
                    for g in (0, 1):
                        emit(q, g, [mk(q, g) for mk in p1], True, not pending)

                def close():
                    if not pending:
                        return
                    for q in range(3):
                        for g in (0, 1):
                            emit(q, g, [mk(q, g) for mk in phase2], False, True)

                return pg, close

            def w_makers(wt, xt, T):
                """bf16 k-pair matmul makers (contraction 256 = 2 passes)."""
                return [
                    (lambda q, g, k=k, w=wt, x=xt: (
                        w[:, k, (2 * q + g) * 128:(2 * q + g + 1) * 128],
                        x[:, k, :],
                        None,
                    ))
                    for k in (0, 1)
                ]

            def precompute_range(makers_fn, pre, lo, hi):
                """pre[:, m, lo:hi] = sum of matmuls over small cols lo:hi."""
                for m in range(6):
                    p = psf.tile([128, hi - lo], f32, name="pp", tag="pf")
                    mms = [mk(m // 2, m % 2) for mk in makers_fn(lo, hi)]
                    for i, (lhs, rhs, pm) in enumerate(mms):
                        nc.tensor.matmul(
                            p, lhs, rhs, start=(i == 0), stop=(i == len(mms) - 1),
                            perf_mode=pm,
                        )
                    nc.vector.tensor_copy(pre[:, m, lo:hi], p)

            def precompute(makers):
                """pre[:, m, 0:SM] = sum of matmuls over the small cols."""
                pre = const.tile([128, 6, SM], f32, name="pre", tag="pre")
                for m in range(6):
                    p = psf.tile([128, SM], f32, name="pp", tag="pf")
                    mms = [mk(m // 2, m % 2) for mk in makers]
                    for i, (lhs, rhs, pm) in enumerate(mms):
                        nc.tensor.matmul(
                            p, lhs, rhs, start=(i == 0), stop=(i == len(mms) - 1),
                            perf_mode=pm,
                        )
                    nc.vector.tensor_copy(pre[:, m, :], p)
                return pre

            pre_td = const.tile([128, 6, SM], f32, name="pre_td", tag="pre2")

            def pre_td_makers(lo, hi):
                return [
                    (lambda q, g, k=k, a=lo, b=hi: (
                        wx16[:, k, (2 * q + g) * 128:(2 * q + g + 1) * 128],
                        xsm[:, k, a:b],
                        None,
                    ))
                    for k in (0, 1)
                ] + [
                    lambda q, g, a=lo, b=hi: (wsl(wh8, q, g), hbu8[:, :, a:b], DR)
                ]

            def gates(pg, T, c_red, c_out, h_out, h8_out=None,
                      leaf_sink=None, pre=None, lev=0):
                """pg: (pgi, pgo, pgu) big tiles or (pio, pu) fused small."""
                fused = len(pg) == 2
                if fused:
                    pio, pu = pg
                    if pre is not None:
                        pv = pio.rearrange("p (m c) -> p m c", m=4)
                        nc.vector.tensor_add(
                            pv, pv, pre[:, 0:4, LEVO[lev]:LEVO[lev] + T]
                        )
                        uv = g2(pu, T)
                        nc.vector.tensor_add(
                            uv, uv, pre[:, 4:6, LEVO[lev]:LEVO[lev] + T]
                        )
                    sio = work.tile([128, 4 * T], bf16, name="sio", tag="ga", bufs=3)
                    nc.scalar.activation(sio, pio, Sig, scale=DS)
                    si, so = sio[:, 0:2 * T], sio[:, 2 * T:4 * T]
                    tu = work.tile([128, 2 * T], bf16, name="tu", tag="gb", bufs=3)
                    nc.scalar.activation(tu, pu, Tanh, scale=DS)
                else:
                    if pre is not None:
                        for q in range(3):
                            pv = g2(pg[q], T)
                            nc.vector.tensor_add(
                                pv, pv, pre[:, 2 * q:2 * q + 2, LEVO[lev]:LEVO[lev] + T]
                            )
                    si = work.tile([128, 2 * T], bf16, name="si", tag="ga", bufs=3)
                    nc.scalar.activation(si, pg[0], Sig, scale=DS)
                    so = work.tile([128, 2 * T], bf16, name="so", tag="gb", bufs=3)
                    nc.scalar.activation(so, pg[1], Sig, scale=DS)
                    tu = work.tile([128, 2 * T], bf16, name="tu", tag="gb", bufs=3)
                    nc.scalar.activation(tu, pg[2], Tanh, scale=DS)
                if c_red is None:
                    nc.vector.tensor_mul(c_out, g2(si, T), g2(tu, T))
                else:
                    nc.vector.tensor_mul(si, si, tu)  # situ, in place
                    kind, cr = c_red
                    if kind == "full":
                        nc.vector.tensor_add(c_out, g2(si, T), cr)
                    else:  # parent-granularity c_red, broadcast to child pairs
                        pT = T // 2
                        si4 = si.rearrange("p (g n two) -> p g n two", g=2, two=2)
                        co4 = c_out.rearrange("p g (n two) -> p g n two", two=2)
                        crb = cr.to_broadcast([128, 2, pT, 2])
                        nc.vector.tensor_add(co4, si4, crb)
                tct = work.tile([128, 2 * T], bf16, name="tct", tag="ga", bufs=3)
                nc.scalar.activation(g2(tct, T), c_out, Tanh)
                if h_out is not None:
                    nc.vector.tensor_mul(h_out, g2(so, T), g2(tct, T))
                    if h8_out is not None:
                        nc.vector.tensor_copy(h8_out, h_out)
                elif h8_out is not None:
                    nc.vector.tensor_mul(h8_out, g2(so, T), g2(tct, T))
                else:
                    ht = work.tile([128, 2 * T], bf16, name="ht", tag="hsum", bufs=3)
                    nc.vector.tensor_mul(g2(ht, T), g2(so, T), g2(tct, T))
                    leaf_sink(ht)

            # ================= bottom-up =================
            filler(16, 256)
            pre_bu = None
            with tc.tile_pool(name="bu_state", bufs=1) as bu_state:
                c_next = None
                C_next = 0
                for l in range(DEPTH, -1, -1):
                    if l < SM_LEV:
                        filler(3)
                    if l == 0:
                        # pre_td for levels 1..5 overlaps the last BU levels
                        precompute_range(pre_td_makers, pre_td, LEVO[1], SM)
                    if l == SM_LEV - 1 and pre_bu is None:
                        load_xsm()
                        load_td_weights()
                        pre_bu = precompute(
                            [
                                (lambda q, g, k=k: (
                                    w16[:, k, (2 * q + g) * 128:(2 * q + g + 1) * 128],
                                    xsm[:, k, 0:SM],
                                    None,
                                ))
                                for k in (0, 1)
                            ]
                        )
                    C, off = LEVW[l], LEVO[l]
                    T = min(TMAX, C)
                    leaf = l == DEPTH
                    small = l < SM_LEV
                    par = "A" if l % 2 else "Bp"
                    c_cur = bu_state.tile([128, 2 * C], bf16, name=f"c{l}", tag=f"c{par}")
                    choff = LEVO[l + 1] if not leaf else 0
                    ntile = C // T
                    # hsum for the whole level up front (bf16, feeds U_bu)
                    hsums = []
                    if not leaf:
                        for j in range(ntile):
                            o0 = j * T
                            hsum = work.tile([128, 2 * T], bf16, name="hsum", tag="hsum", bufs=3)
                            cb = choff + 2 * o0
                            hb4 = hbu.rearrange("p (k c) -> p k c", k=2)[
                                :, :, cb:cb + 2 * T
                            ].rearrange("p k (n two) -> p k n two", two=2)
                            nc.vector.tensor_add(
                                g2(hsum, T), hb4[:, :, :, 0], hb4[:, :, :, 1]
                            )
                            hsums.append(hsum)
                    for j in range(ntile):
                        o0 = j * T
                        xt = None if small else load_x(off, o0, T)
                        cred = None
                        u_phase = None
                        pg = close = None
                        if not leaf:
                            ncj = 2 if 2 * T > TMAX else 1
                            Tc = 2 * T // ncj
                            cred = work.tile([128, 2 * T], bf16, name="cred", tag="cred")
                            hs_ = hsums[j]
                            u_phase = [
                                (lambda q, g, k=k, h=hs_, T_=T: (
                                    ub16[:, k, (2 * q + g) * 128:(2 * q + g + 1) * 128],
                                    h[:, k * T_:(k + 1) * T_],
                                    None,
                                ))
                                for k in (0, 1)
                            ]
                        if not small:
                            pg, close = iou_mms(T, w_makers(w16, xt, T), u_phase)
                        if not leaf:
                            for cj in range(ncj):
                                cb = choff + 2 * o0 + cj * Tc
                                pf = psf.tile([128, 2 * Tc], f32, name="pf", tag="pf")
                                for g in (0, 1):
                                    nc.tensor.matmul(
                                        pf[:, g * Tc:(g + 1) * Tc],
                                        wsl(ufb8, 0, g),
                                        hbu8[:, :, cb:cb + Tc],
                                        start=True, stop=True, perf_mode=DR,
                                    )
                                # f out of PSUM at the act (bank recycles now)
                                fs = work.tile([128, 2 * Tc], bf16, name="fs", tag="fs", bufs=3)
                                nc.scalar.activation(fs, pf, Sig, scale=DS)
                                cv = g2(c_next, C_next)[
                                    :, :, 2 * o0 + cj * Tc:2 * o0 + (cj + 1) * Tc
                                ]
                                fct = work.tile([128, 2 * Tc], bf16, name="fct", tag="fc")
                                nc.vector.tensor_mul(g2(fct, Tc), g2(fs, Tc), cv)
                                h2 = Tc // 2
                                crv = g2(cred, T)[:, :, cj * h2:(cj + 1) * h2]
                                fc4 = fct.rearrange("p (g n two) -> p g n two", g=2, two=2)
                                eng = nc.vector if small else nc.gpsimd
                                eng.tensor_add(crv, fc4[:, :, :, 0], fc4[:, :, :, 1])
                        if small:
                            pg, close = iou_mms(T, u_phase)
                        else:
                            close()
                        cr = None if leaf else ("full", g2(cred, T))
                        gates(
                            pg, T, cr,
                            g2(c_cur, C)[:, :, o0:o0 + T],
                            hbu.rearrange("p (k c) -> p k c", k=2)[
                                :, :, off + o0:off + o0 + T
                            ],
                            h8_out=hbu8[:, :, off + o0:off + o0 + T],
                            pre=pre_bu if small else None,
                            lev=l,
                        )
                    c_next = c_cur
                    C_next = C

            # root h_bu staged to f32 for the output DMA
            nc.vector.tensor_copy(
                root_f32, hbu.rearrange("p (k c) -> p k c", k=2)[:, :, 0:bl]
            )

            # ---- pre_td root columns (needs the last BU output) ----
            precompute_range(pre_td_makers, pre_td, 0, LEVO[1])

            # ================= top-down =================
            with tc.tile_pool(name="td_state", bufs=1) as td_state:
                h_prev = c_prev = None
                C_prev = 0
                for l in range(0, DEPTH + 1):
                    C, off = LEVW[l], LEVO[l]
                    T = min(TMAX, C)
                    leaf = l == DEPTH
                    root = l == 0
                    small = l < SM_LEV
                    par = "A" if l % 2 else "Bp"
                    if not leaf:
                        Cp = max(C, 16)  # fp8 pair-dim stride must be %16
                        h_cur = td_state.tile([128, 2, Cp], f8, name=f"th{l}", tag=f"th{par}")
                        c_cur = td_state.tile([128, 2 * C], bf16, name=f"tc{l}", tag=f"tc{par}")
                    else:
                        h_cur = c_cur = None
                    if small:
                        filler(3)
                    for j in range(C // T):
                        o0 = j * T
                        xt = None if small else load_x(off, o0, T)
                        credp = None
                        pT = T // 2 if not root else 0
                        po = o0 // 2
                        u_phase = None
                        if not root:
                            hp_, po_, pT_ = h_prev, po, pT
                            u_phase = [
                                lambda q, g, h=hp_, a=po_, b=pT_: (
                                    wsl(ut8, q, g),
                                    h[:, :, a:a + b].to_broadcast([128, 2, b, 2]),
                                    DR,
                                )
                            ]
                        if not small:
                            ph1 = w_makers(wx16, xt, T) + [
                                lambda q, g, a=off + o0, T_=T: (
                                    wsl(wh8, q, g), hbu8[:, :, a:a + T_], DR)
                            ]
                            pg, close = iou_mms(T, ph1, u_phase)
                        if not root:
                            pf = psf.tile([128, 2 * pT], f32, name="pftd", tag="pf")
                            for g in (0, 1):
                                nc.tensor.matmul(
                                    pf[:, g * pT:(g + 1) * pT],
                                    wsl(uft8, 0, g),
                                    h_prev[:, :, po:po + pT],
                                    start=True, stop=True, perf_mode=DR,
                                )
                            fs = work.tile([128, 2 * pT], bf16, name="fstd", tag="fs", bufs=3)
                            nc.scalar.activation(fs, pf, Sig, scale=DS)
                            credp = work.tile([128, 2 * pT], bf16, name="credp", tag="cred")
                            nc.vector.tensor_mul(
                                g2(credp, pT), g2(fs, pT),
                                g2(c_prev, C_prev)[:, :, po:po + pT],
                            )
                        if small:
                            pg, close = iou_mms(T, u_phase or [])
                        else:
                            close()
                        if root:
                            pio, pu = pg
                            nc.vector.tensor_copy(
                                pio.rearrange("p (m c) -> p m c", m=4),
                                pre_td[:, 0:4, 0:T],
                            )
                            nc.vector.tensor_copy(g2(pu, T), pre_td[:, 4:6, 0:T])
                            prearg = None
                        else:
                            prearg = pre_td if small else None
                        if leaf:
                            cl = work.tile([128, 2 * T], bf16, name="cl", tag="fc")
                            tree = j

                            def sink(ht, _t=tree):
                                for g in (0, 1):
                                    nc.vector.reduce_sum(
                                        mean[:, g, _t:_t + 1],
                                        ht[:, g * T:(g + 1) * T],
                                        axis=mybir.AxisListType.X,
                                    )

                            gates(
                                pg, T, ("parent", g2(credp, pT)),
                                g2(cl, T), None, leaf_sink=sink,
                                pre=prearg, lev=l,
                            )
                        else:
                            gates(
                                pg, T,
                                None if root else ("parent", g2(credp, pT)),
                                g2(c_cur, C)[:, :, o0:o0 + T],
                                None,
                                h8_out=h_cur[:, :, o0:o0 + T],
                                pre=prearg, lev=l,
                            )
                    h_prev, c_prev = h_cur, c_cur
                    C_prev = C

            # ---- outputs ----
            mf = mean.rearrange("p g b -> p (g b)")
            nc.vector.tensor_scalar_mul(mf, mf, 1.0 / (1 << DEPTH))
            nc.sync.dma_start(
                out=out_d[0:256, :].rearrange("(k p) b -> p k b", k=2),
                in_=root_f32,
            )
            nc.sync.dma_start(
                out=out_d[256:512, :].rearrange("(g p) b -> p g b", g=2),
                in_=mean,
            )

    if not nc.is_finalized():
        nc.finalize()
    return nc


def _q8(x, scale):
    return np.clip(np.asarray(x, np.float32) * scale, -240, 240).astype(E4)


def _prep_shared(inputs):
    """Host-side weight marshaling (biases are zero by spec)."""
    f = np.ascontiguousarray
    W_iou_td = np.asarray(inputs["W_iou_td"], np.float32)
    w_bu = np.asarray(inputs["W_iou_bu"], np.float32).T  # [256, 768]
    wx_td = W_iou_td[:, :XS].T
    wh_td = f(W_iou_td[:, XS:].T)
    u_td = np.asarray(inputs["U_iou_td"], np.float32).T
    uf_bu = np.asarray(inputs["U_f_bu"], np.float32).T
    uf_td = np.asarray(inputs["U_f_td"], np.float32).T
    u_bu = np.asarray(inputs["U_iou_bu"], np.float32).T
    return {
        "w_bu16": f((w_bu * 256.0).astype(BF16)),
        "u_bu16": f((u_bu * 256.0).astype(BF16)),
        "wx_td16": f((wx_td * 256.0).astype(BF16)),
        "wh_td8": f(_q8(wh_td, 256.0)),
        "u_td8": f(_q8(u_td, 256.0)),
        "uf_bu8": f(_q8(uf_bu, 256.0)),
        "uf_td8": f(_q8(uf_td, 256.0)),
    }


def prep_xt(Xc):
    """[bl, NN, XS] -> level-major [256, bl*NN] bf16."""
    bl = Xc.shape[0]
    xt = np.asarray(Xc, np.float32).transpose(2, 0, 1)  # [XS, bl, NN]
    blocks = []
    for l in range(DEPTH + 1):
        lo, nl = (1 << l) - 1, 1 << l
        blocks.append(xt[:, :, lo:lo + nl].reshape(XS, bl * nl))
    return np.ascontiguousarray(np.concatenate(blocks, axis=1).astype(BF16))


def unpack_out(o, bl):
    """[512, bl] -> [bl, 512] (root_h_bu | leaf mean)."""
    return np.concatenate([o[0:256, :].T, o[256:512, :].T], axis=1)


def kernel(**inputs):
    global LAST_EXEC_NS
    from concourse.bass_utils import run_bass_kernel_spmd

    bl = B // NCORES
    if "nc" not in _CACHE:
        _CACHE["nc"] = _build_nc(bl)
    nc = _CACHE["nc"]

    shared = _prep_shared(inputs)
    X = np.asarray(inputs["X"], np.float32)
    in_maps = []
    for c in range(NCORES):
        m = dict(shared)
        m["x16"] = prep_xt(X[c * bl:(c + 1) * bl])
        in_maps.append(m)

    trace = _CACHE.get("trace", False)
    res = None
    for attempt in range(3):
        try:
            res = run_bass_kernel_spmd(nc, in_maps, list(range(NCORES)), trace=trace)
            break
        except Exception:
            if attempt == 2:
                raise
            import time

            time.sleep(5)
    LAST_EXEC_NS = res.exec_time_ns
    _CACHE["last_results"] = res

    out = np.concatenate(
        [unpack_out(res.results[c]["out"], bl) for c in range(NCORES)], axis=0
    )
    return out.astype(np.float32)


# revision 16
# speedup vs baseline: 1.0302x; 1.0302x over previous
"""BiDiTreeLSTM Trainium2 kernel.

Full-input contract: kernel(**inputs) takes the unsharded numpy inputs of
reference.setup_inputs() and returns the full [64, 512] output.

Strategy: data-parallel over trees (8 trees per NeuronCore, 8 cores).
Per-core layout is feature-major: node-state tensors live in SBUF as
[128 partitions, 2 feature halves, cols], cols level-major (tree-major
within a level); children of parent column c are columns 2c/2c+1 of the
next level.

Precision plan (validated vs the jax reference in numpy sim; combined rel
err ~6e-3 against the 2e-2 gate):
 - X-projections (W_iou_bu@X, Wx_td@X) and U_iou_bu@hsum: bf16 operands
   (X plain, weights host-scaled x256).  bf16 matmul = 1 cycle/row at any
   moving dim (fp32r needs >=256), FWL halves LDWEIGHTS, X DMA halves.
 - Wh_td@h_bu, U_iou_td@h_td, U_f_{bu,td}@h: fp8e4 DoubleRow (one matmul
   per 256-contraction instead of two; 0.5 cycles/row).  Weights host-
   scaled x256; h shadows are UNSCALED plain fp8 casts (sim: scaled vs
   unscaled identical).  fp8 on U_iou_bu would fail the accuracy gate
   (2e-2) so that path stays bf16.
 - Every gate/f psum is x256; 1/256 is folded into the activation scale.
h_bu is bf16 (feeds bf16 hsum + f32-staged root output) plus an fp8 shadow
for DoubleRow consumers; TD h state is fp8-only.  fp8 DoubleRow APs keep
the feature-pair dim stride a multiple of 16 bytes.

Zero-fills exploited per the problem spec: h0 == c0 == 0, all biases == 0.
"""

import numpy as np
import ml_dtypes

B, NN, XS, H = 64, 1023, 256, 256
NCORES = 8
DEPTH = 9  # levels 0..9, level l has 2^l nodes per tree
TMAX = 512
SM_LEV = 6  # levels 0..SM_LEV-1 get batched W-projections (pre_bu / pre_td)
TOTP = 8192  # hbu8 padded col count (stride % 16 == 0)

E4 = ml_dtypes.float8_e4m3fn
BF16 = ml_dtypes.bfloat16

_CACHE = {}

LAST_EXEC_NS = None


def _levels(bl):
    levw = [bl * (1 << l) for l in range(DEPTH + 1)]
    levo = [bl * ((1 << l) - 1) for l in range(DEPTH + 1)]
    tot = bl * NN
    return levw, levo, tot


def _build_nc(bl):
    from concourse import bacc
    import concourse.mybir as mybir
    import concourse.tile as tile

    f32 = mybir.dt.float32
    bf16 = mybir.dt.bfloat16
    f8 = mybir.dt.float8e4
    DR = mybir.MatmulPerfMode.DoubleRow
    Sig = mybir.ActivationFunctionType.Sigmoid
    Tanh = mybir.ActivationFunctionType.Tanh
    DS = 1.0 / 256.0  # descale folded into the psum-reading activations

    LEVW, LEVO, TOT = _levels(bl)
    SM = LEVO[SM_LEV]  # cols of levels 0..SM_LEV-1 (contiguous, level-major)
    SMP = 512  # padded stride for the xsm tile

    nc = bacc.Bacc("TRN2", target_bir_lowering=False)

    x16_d = nc.declare_dram_parameter("x16", [256, TOT], bf16, isOutput=False)
    id_d = nc.declare_dram_parameter("ident16", [128, 128], bf16, isOutput=False)
    w16_d = nc.declare_dram_parameter("w_bu16", [256, 3 * H], bf16, isOutput=False)
    ub_d = nc.declare_dram_parameter("u_bu16", [256, 3 * H], bf16, isOutput=False)
    wx16_d = nc.declare_dram_parameter("wx_td16", [256, 3 * H], bf16, isOutput=False)
    wh8_d = nc.declare_dram_parameter("wh_td8", [256, 3 * H], f8, isOutput=False)
    ut8_d = nc.declare_dram_parameter("u_td8", [256, 3 * H], f8, isOutput=False)
    ufb8_d = nc.declare_dram_parameter("uf_bu8", [256, H], f8, isOutput=False)
    uft8_d = nc.declare_dram_parameter("uf_td8", [256, H], f8, isOutput=False)
    out_d = nc.declare_dram_parameter("out", [512, bl], f32, isOutput=True)

    with tile.TileContext(nc) as tc:
        with (
            tc.tile_pool(name="const", bufs=1) as const,
            tc.tile_pool(name="hbu_pool", bufs=1) as hbu_pool,
            tc.tile_pool(name="work", bufs=2) as work,
            tc.tile_pool(name="xtp", bufs=3) as xtp,
            tc.tile_pool(name="psg", bufs=1, space="PSUM") as psg,
            tc.tile_pool(name="psf", bufs=1, space="PSUM") as psf,
        ):
            def load_16(dram, cols, nm, eng):
                t = const.tile([128, 2, cols], bf16, name=nm, tag=nm)
                for k in (0, 1):
                    eng.dma_start(out=t[:, k, :], in_=dram[k * 128:(k + 1) * 128, :])
                return t

            def load_8(dram, cols, nm, eng):
                t = const.tile([128, 2, cols], f8, name=nm, tag=nm)
                for k in (0, 1):
                    eng.dma_start(out=t[:, k, :], in_=dram[k * 128:(k + 1) * 128, :])
                return t

            # bu weights first: w16 on the sync queue ahead of the xt
            # stream (mm#0 needs it); ub16/ufb8 on scalar (acts start later)
            w16 = load_16(w16_d, 3 * H, "w16", nc.scalar)
            ident = const.tile([128, 128], bf16, name="ident", tag="ident")
            nc.scalar.dma_start(out=ident, in_=id_d[:, :])
            ub16 = load_16(ub_d, 3 * H, "ub16", nc.scalar)
            ufb8 = load_8(ufb8_d, H, "ufb8", nc.scalar)
            # td weights are DMA'd later (emitted after the BU leaf level)
            wx16 = const.tile([128, 2, 3 * H], bf16, name="wx16", tag="wx16")
            wh8 = const.tile([128, 2, 3 * H], f8, name="wh8", tag="wh8")
            ut8 = const.tile([128, 2, 3 * H], f8, name="ut8", tag="ut8")
            uft8 = const.tile([128, 2, H], f8, name="uft8", tag="uft8")

            def load_td_weights():
                # sync queue is idle once the last BU xt tiles are in flight
                for k in (0, 1):
                    nc.sync.dma_start(out=wx16[:, k, :], in_=wx16_d[k * 128:(k + 1) * 128, :])
                    nc.sync.dma_start(out=wh8[:, k, :], in_=wh8_d[k * 128:(k + 1) * 128, :])
                    nc.sync.dma_start(out=ut8[:, k, :], in_=ut8_d[k * 128:(k + 1) * 128, :])
                    nc.sync.dma_start(out=uft8[:, k, :], in_=uft8_d[k * 128:(k + 1) * 128, :])

            # h_bu: bf16 primary (hsum, root out) + fp8 shadow (DR rhs)
            hbu = hbu_pool.tile([128, 2 * TOT], bf16, name="hbu", tag="hbu")
            hbu8 = hbu_pool.tile([128, 2, TOTP], f8, name="hbu8", tag="hbu8")
            mean = const.tile([128, 2, bl], f32, name="mean", tag="mean")
            root_f32 = const.tile([128, 2, bl], f32, name="rootf", tag="rootf")

            xsm = const.tile([128, 2, SMP], bf16, name="xsm", tag="xsm")

            # HAM-warming filler: dead matmuls on a scratch tile keep the PE
            # activity window busy through DMA startup and the tiny levels,
            # so real matmuls run at 2.4 GHz instead of the cold 1.2.
            junk = const.tile([128, 2, 512], bf16, name="junk", tag="junk")
            nc.gpsimd.memset(junk, 0.0)

            def filler(n, cols=512):
                for _ in range(n):
                    pj = psf.tile([128, cols], f32, name="junk_ps", tag="pf")
                    nc.tensor.matmul(
                        pj, junk[:, 0, 0:128], junk[:, 1, 0:cols],
                        start=True, stop=True,
                    )

            def load_xsm():
                for k in (0, 1):
                    nc.sync.dma_start(out=xsm[:, k, 0:SM], in_=x16_d[k * 128:(k + 1) * 128, 0:SM])

            def load_x(off, o0, T):
                xt = xtp.tile([128, 2, T], bf16, name="xt", tag="xt", bufs=3)
                for k in (0, 1):
                    nc.sync.dma_start(out=xt[:, k, :], in_=x16_d[k * 128:(k + 1) * 128, off + o0:off + o0 + T])
                return xt

            def g2(ap, width):
                return ap.rearrange("p (g c) -> p g c", g=2)

            def wsl(wt, q, g):
                m = 2 * q + g
                return wt[:, :, m * 128:(m + 1) * 128]

            # ---- per-tile gate matmuls -------------------------------------
            # Three per-gate psum tiles [128, 2T] (tags pgi/pgo/pgu, bufs=1):
            # the next tile's gate-q matmuls wait only on this tile's gate-q
            # activation.  phase1 = input-ready matmuls, phase2 = recurrence-
            # dependent ones, deferred (groups left open) when each T slice
            # is a whole PSUM bank (T == 512), else merged.
            def iou_mms(T, phase1, phase2=None):
                """Small levels (T <= 256) share one [128, 4T] i|o psum tile
                (tag pgi, same banks) so a single fused Sigmoid evacuates
                both gates; big tiles keep three per-gate tiles for bank-
                level pipelining with deferred phase2."""
                fused = T <= 256
                merged = phase2 is not None and (T < 512 or fused)
                p1 = phase1 + phase2 if merged else phase1
                pending = phase2 is not None and not merged
                if fused:
                    pio = psg.tile([128, 4 * T], f32, name="pio", tag="pgi")
                    pu = psg.tile([128, 2 * T], f32, name="pu", tag="pgu")
                    pg = (pio, pu)

                    def slot(q, g):
                        if q < 2:
                            return pio[:, (2 * q + g) * T:(2 * q + g + 1) * T]
                        return pu[:, g * T:(g + 1) * T]
                else:
                    tiles = []
                    for tag in ("pgi", "pgo", "pgu"):
                        tiles.append(psg.tile([128, 2 * T], f32, name=tag, tag=tag))
                    pg = tuple(tiles)

                    def slot(q, g):
                        return pg[q][:, g * T:(g + 1) * T]

                def emit(q, g, mms, start, stop):
                    for i, (lhs, rhs, pm) in enumerate(mms):
                        nc.tensor.matmul(
                            slot(q, g), lhs, rhs,
                            start=(start and i == 0),
                            stop=(stop and i == len(mms) - 1),
                            perf_mode=pm,
                        )

                for q in range(3):
                    for g in (0, 1):
                        emit(q, g, [mk(q, g) for mk in p1], True, not pending)

                def close():
                    if not pending:
                        return
                    for q in range(3):
                        for g in (0, 1):
                            emit(q, g, [mk(q, g) for mk in phase2], False, True)

                return pg, close

            def w_makers(wt, xt, T):
                """bf16 k-pair matmul makers (contraction 256 = 2 passes)."""
                return [
                    (lambda q, g, k=k, w=wt, x=xt: (
                        w[:, k, (2 * q + g) * 128:(2 * q + g + 1) * 128],
                        x[:, k, :],
                        None,
                    ))
                    for k in (0, 1)
                ]

            def precompute_range(makers_fn, pre, lo, hi):
                """pre[:, m, lo:hi] = sum of matmuls over small cols lo:hi."""
                for m in range(6):
                    p = psf.tile([128, hi - lo], f32, name="pp", tag="pf")
                    mms = [mk(m // 2, m % 2) for mk in makers_fn(lo, hi)]
                    for i, (lhs, rhs, pm) in enumerate(mms):
                        nc.tensor.matmul(
                            p, lhs, rhs, start=(i == 0), stop=(i == len(mms) - 1),
                            perf_mode=pm,
                        )
                    nc.vector.tensor_copy(pre[:, m, lo:hi], p)

            pre_td = const.tile([128, 6, SM], bf16, name="pre_td", tag="pre2")

            def pre_maker(pre, lev):
                """identity matmul injecting pre[:, 2q+g, cols] into the
                psum group -- keeps the whole small-level chain PE->scalar."""
                return lambda q, g, p=pre, l=lev: (
                    ident,
                    p[:, 2 * q + g, LEVO[l]:LEVO[l] + LEVW[l]],
                    None,
                )

            def pre_td_makers(lo, hi):
                return [
                    (lambda q, g, k=k, a=lo, b=hi: (
                        wx16[:, k, (2 * q + g) * 128:(2 * q + g + 1) * 128],
                        xsm[:, k, a:b],
                        None,
                    ))
                    for k in (0, 1)
                ] + [
                    lambda q, g, a=lo, b=hi: (wsl(wh8, q, g), hbu8[:, :, a:b], DR)
                ]

            def gates(pg, T, c_red, c_out, h_out, h8_out=None,
                      leaf_sink=None, pre=None, lev=0):
                """pg: (pgi, pgo, pgu) big tiles or (pio, pu) fused small."""
                fused = len(pg) == 2
                if fused:
                    pio, pu = pg
                    sio = work.tile([128, 4 * T], bf16, name="sio", tag="ga", bufs=3)
                    nc.scalar.activation(sio, pio, Sig, scale=DS)
                    si, so = sio[:, 0:2 * T], sio[:, 2 * T:4 * T]
                    tu = work.tile([128, 2 * T], bf16, name="tu", tag="gb", bufs=3)
                    nc.scalar.activation(tu, pu, Tanh, scale=DS)
                else:
                    si = work.tile([128, 2 * T], bf16, name="si", tag="ga", bufs=3)
                    nc.scalar.activation(si, pg[0], Sig, scale=DS)
                    so = work.tile([128, 2 * T], bf16, name="so", tag="gb", bufs=3)
                    nc.scalar.activation(so, pg[1], Sig, scale=DS)
                    tu = work.tile([128, 2 * T], bf16, name="tu", tag="gb", bufs=3)
                    nc.scalar.activation(tu, pg[2], Tanh, scale=DS)
                if c_red is None:
                    nc.vector.tensor_mul(c_out, g2(si, T), g2(tu, T))
                else:
                    nc.vector.tensor_mul(si, si, tu)  # situ, in place
                    kind, cr = c_red
                    if kind == "full":
                        nc.vector.tensor_add(c_out, g2(si, T), cr)
                    else:  # parent-granularity c_red, broadcast to child pairs
                        pT = T // 2
                        si4 = si.rearrange("p (g n two) -> p g n two", g=2, two=2)
                        co4 = c_out.rearrange("p g (n two) -> p g n two", two=2)
                        crb = cr.to_broadcast([128, 2, pT, 2])
                        nc.vector.tensor_add(co4, si4, crb)
                tct = work.tile([128, 2 * T], bf16, name="tct", tag="ga", bufs=3)
                nc.scalar.activation(g2(tct, T), c_out, Tanh)
                if h_out is not None:
                    nc.vector.tensor_mul(h_out, g2(so, T), g2(tct, T))
                    if h8_out is not None:
                        nc.vector.tensor_copy(h8_out, h_out)
                elif h8_out is not None:
                    nc.vector.tensor_mul(h8_out, g2(so, T), g2(tct, T))
                else:
                    ht = work.tile([128, 2 * T], bf16, name="ht", tag="hsum", bufs=3)
                    nc.vector.tensor_mul(g2(ht, T), g2(so, T), g2(tct, T))
                    leaf_sink(ht)

            # ================= bottom-up =================
            filler(16, 256)
            load_xsm()
            pre_bu = const.tile([128, 6, SM], bf16, name="pre", tag="pre")
            pre_bu_makers = lambda lo, hi: [
                (lambda q, g, k=k, a=lo, b=hi: (
                    w16[:, k, (2 * q + g) * 128:(2 * q + g + 1) * 128],
                    xsm[:, k, a:b],
                    None,
                ))
                for k in (0, 1)
            ]
            with tc.tile_pool(name="bu_state", bufs=1) as bu_state:
                c_next = None
                C_next = 0
                for l in range(DEPTH, -1, -1):
                    if l == 0:
                        # pre_td for levels 1..5 overlaps the last BU levels
                        precompute_range(pre_td_makers, pre_td, LEVO[1], SM)
                    if l == SM_LEV - 1:
                        load_td_weights()
                    C, off = LEVW[l], LEVO[l]
                    T = min(TMAX, C)
                    leaf = l == DEPTH
                    small = l < SM_LEV
                    par = "A" if l % 2 else "Bp"
                    c_cur = bu_state.tile([128, 2 * C], bf16, name=f"c{l}", tag=f"c{par}")
                    choff = LEVO[l + 1] if not leaf else 0
                    ntile = C // T
                    # hsum for the whole level up front (bf16, feeds U_bu)
                    hsums = []
                    if not leaf:
                        for j in range(ntile):
                            o0 = j * T
                            hsum = work.tile([128, 2 * T], bf16, name="hsum", tag="hsum", bufs=3)
                            cb = choff + 2 * o0
                            hb4 = hbu.rearrange("p (k c) -> p k c", k=2)[
                                :, :, cb:cb + 2 * T
                            ].rearrange("p k (n two) -> p k n two", two=2)
                            nc.vector.tensor_add(
                                g2(hsum, T), hb4[:, :, :, 0], hb4[:, :, :, 1]
                            )
                            hsums.append(hsum)
                    for j in range(ntile):
                        if leaf and j == 2:
                            precompute_range(pre_bu_makers, pre_bu, 0, SM)
                        o0 = j * T
                        xt = None if small else load_x(off, o0, T)
                        cred = None
                        u_phase = None
                        pg = close = None
                        if not leaf:
                            ncj = 2 if 2 * T > TMAX else 1
                            Tc = 2 * T // ncj
                            cred = work.tile([128, 2 * T], bf16, name="cred", tag="cred")
                            hs_ = hsums[j]
                            u_phase = [
                                (lambda q, g, k=k, h=hs_, T_=T: (
                                    ub16[:, k, (2 * q + g) * 128:(2 * q + g + 1) * 128],
                                    h[:, k * T_:(k + 1) * T_],
                                    None,
                                ))
                                for k in (0, 1)
                            ]
                        if not small:
                            pg, close = iou_mms(T, w_makers(w16, xt, T), u_phase)
                        if not leaf:
                            for cj in range(ncj):
                                cb = choff + 2 * o0 + cj * Tc
                                pf = psf.tile([128, 2 * Tc], f32, name="pf", tag="pf")
                                for g in (0, 1):
                                    nc.tensor.matmul(
                                        pf[:, g * Tc:(g + 1) * Tc],
                                        wsl(ufb8, 0, g),
                                        hbu8[:, :, cb:cb + Tc],
                                        start=True, stop=True, perf_mode=DR,
                                    )
                                # f out of PSUM at the act (bank recycles now)
                                fs = work.tile([128, 2 * Tc], bf16, name="fs", tag="fs", bufs=3)
                                nc.scalar.activation(fs, pf, Sig, scale=DS)
                                cv = g2(c_next, C_next)[
                                    :, :, 2 * o0 + cj * Tc:2 * o0 + (cj + 1) * Tc
                                ]
                                fct = work.tile([128, 2 * Tc], bf16, name="fct", tag="fc")
                                nc.vector.tensor_mul(g2(fct, Tc), g2(fs, Tc), cv)
                                h2 = Tc // 2
                                crv = g2(cred, T)[:, :, cj * h2:(cj + 1) * h2]
                                fc4 = fct.rearrange("p (g n two) -> p g n two", g=2, two=2)
                                eng = nc.vector if small else nc.gpsimd
                                eng.tensor_add(crv, fc4[:, :, :, 0], fc4[:, :, :, 1])
                        if small:
                            pg, close = iou_mms(T, [pre_maker(pre_bu, l)] + u_phase)
                        else:
                            close()
                        cr = None if leaf else ("full", g2(cred, T))
                        gates(
                            pg, T, cr,
                            g2(c_cur, C)[:, :, o0:o0 + T],
                            hbu.rearrange("p (k c) -> p k c", k=2)[
                                :, :, off + o0:off + o0 + T
                            ],
                            h8_out=hbu8[:, :, off + o0:off + o0 + T],
                            lev=l,
                        )
                    c_next = c_cur
                    C_next = C

            # root h_bu staged to f32 and shipped right away
            nc.vector.tensor_copy(
                root_f32, hbu.rearrange("p (k c) -> p k c", k=2)[:, :, 0:bl]
            )
            nc.sync.dma_start(
                out=out_d[0:256, :].rearrange("(k p) b -> p k b", k=2),
                in_=root_f32,
            )

            # ---- pre_td root columns (needs the last BU output) ----
            precompute_range(pre_td_makers, pre_td, 0, LEVO[1])

            # ================= top-down =================
            with tc.tile_pool(name="td_state", bufs=1) as td_state:
                h_prev = c_prev = None
                C_prev = 0
                for l in range(0, DEPTH + 1):
                    C, off = LEVW[l], LEVO[l]
                    T = min(TMAX, C)
                    leaf = l == DEPTH
                    root = l == 0
                    small = l < SM_LEV
                    par = "A" if l % 2 else "Bp"
                    if not leaf:
                        Cp = max(C, 16)  # fp8 pair-dim stride must be %16
                        h_cur = td_state.tile([128, 2, Cp], f8, name=f"th{l}", tag=f"th{par}")
                        c_cur = td_state.tile([128, 2 * C], bf16, name=f"tc{l}", tag=f"tc{par}")
                    else:
                        h_cur = c_cur = None
                    for j in range(C // T):
                        o0 = j * T
                        xt = None if small else load_x(off, o0, T)
                        credp = None
                        pT = T // 2 if not root else 0
                        po = o0 // 2
                        u_phase = None
                        if not root:
                            hp_, po_, pT_ = h_prev, po, pT
                            u_phase = [
                                lambda q, g, h=hp_, a=po_, b=pT_: (
                                    wsl(ut8, q, g),
                                    h[:, :, a:a + b].to_broadcast([128, 2, b, 2]),
                                    DR,
                                )
                            ]
                        if not small:
                            ph1 = w_makers(wx16, xt, T) + [
                                lambda q, g, a=off + o0, T_=T: (
                                    wsl(wh8, q, g), hbu8[:, :, a:a + T_], DR)
                            ]
                            pg, close = iou_mms(T, ph1, u_phase)
                        if not root:
                            pf = psf.tile([128, 2 * pT], f32, name="pftd", tag="pf")
                            for g in (0, 1):
                                nc.tensor.matmul(
                                    pf[:, g * pT:(g + 1) * pT],
                                    wsl(uft8, 0, g),
                                    h_prev[:, :, po:po + pT],
                                    start=True, stop=True, perf_mode=DR,
                                )
                            fs = work.tile([128, 2 * pT], bf16, name="fstd", tag="fs", bufs=3)
                            nc.scalar.activation(fs, pf, Sig, scale=DS)
                            credp = work.tile([128, 2 * pT], bf16, name="credp", tag="cred")
                            nc.vector.tensor_mul(
                                g2(credp, pT), g2(fs, pT),
                                g2(c_prev, C_prev)[:, :, po:po + pT],
                            )
                        if small:
                            pg, close = iou_mms(T, [pre_maker(pre_td, l)] + (u_phase or []))
                        else:
                            close()
                        if leaf:
                            cl = work.tile([128, 2 * T], bf16, name="cl", tag="fc")
                            tree = j

                            def sink(ht, _t=tree):
                                for g in (0, 1):
                                    nc.vector.reduce_sum(
                                        mean[:, g, _t:_t + 1],
                                        ht[:, g * T:(g + 1) * T],
                                        axis=mybir.AxisListType.X,
                                    )
                                mv = mean[:, :, _t:_t + 1]
                                nc.vector.tensor_scalar_mul(
                                    mv, mv, 1.0 / (1 << DEPTH)
                                )
                                nc.sync.dma_start(
                                    out=out_d[256:512, _t:_t + 1].rearrange(
                                        "(g p) b -> p g b", g=2
                                    ),
                                    in_=mv,
                                )

                            gates(
                                pg, T, ("parent", g2(credp, pT)),
                                g2(cl, T), None, leaf_sink=sink,
                            )
                        else:
                            gates(
                                pg, T,
                                None if root else ("parent", g2(credp, pT)),
                                g2(c_cur, C)[:, :, o0:o0 + T],
                                None,
                                h8_out=h_cur[:, :, o0:o0 + T],
                            )
                    h_prev, c_prev = h_cur, c_cur
                    C_prev = C


    if not nc.is_finalized():
        nc.finalize()
    return nc


def _q8(x, scale):
    return np.clip(np.asarray(x, np.float32) * scale, -240, 240).astype(E4)


def _prep_shared(inputs):
    """Host-side weight marshaling (biases are zero by spec)."""
    f = np.ascontiguousarray
    W_iou_td = np.asarray(inputs["W_iou_td"], np.float32)
    w_bu = np.asarray(inputs["W_iou_bu"], np.float32).T  # [256, 768]
    wx_td = W_iou_td[:, :XS].T
    wh_td = f(W_iou_td[:, XS:].T)
    u_td = np.asarray(inputs["U_iou_td"], np.float32).T
    uf_bu = np.asarray(inputs["U_f_bu"], np.float32).T
    uf_td = np.asarray(inputs["U_f_td"], np.float32).T
    u_bu = np.asarray(inputs["U_iou_bu"], np.float32).T
    return {
        "ident16": np.eye(128, dtype=np.float32).astype(BF16),
        "w_bu16": f((w_bu * 256.0).astype(BF16)),
        "u_bu16": f((u_bu * 256.0).astype(BF16)),
        "wx_td16": f((wx_td * 256.0).astype(BF16)),
        "wh_td8": f(_q8(wh_td, 256.0)),
        "u_td8": f(_q8(u_td, 256.0)),
        "uf_bu8": f(_q8(uf_bu, 256.0)),
        "uf_td8": f(_q8(uf_td, 256.0)),
    }


def prep_xt(Xc):
    """[bl, NN, XS] -> level-major [256, bl*NN] bf16."""
    bl = Xc.shape[0]
    xt = np.asarray(Xc, np.float32).transpose(2, 0, 1)  # [XS, bl, NN]
    blocks = []
    for l in range(DEPTH + 1):
        lo, nl = (1 << l) - 1, 1 << l
        blocks.append(xt[:, :, lo:lo + nl].reshape(XS, bl * nl))
    return np.ascontiguousarray(np.concatenate(blocks, axis=1).astype(BF16))


def unpack_out(o, bl):
    """[512, bl] -> [bl, 512] (root_h_bu | leaf mean)."""
    return np.concatenate([o[0:256, :].T, o[256:512, :].T], axis=1)


def kernel(**inputs):
    global LAST_EXEC_NS
    from concourse.bass_utils import run_bass_kernel_spmd

    bl = B // NCORES
    if "nc" not in _CACHE:
        _CACHE["nc"] = _build_nc(bl)
    nc = _CACHE["nc"]

    shared = _prep_shared(inputs)
    X = np.asarray(inputs["X"], np.float32)
    in_maps = []
    for c in range(NCORES):
        m = dict(shared)
        m["x16"] = prep_xt(X[c * bl:(c + 1) * bl])
        in_maps.append(m)

    trace = _CACHE.get("trace", False)
    res = None
    for attempt in range(3):
        try:
            res = run_bass_kernel_spmd(nc, in_maps, list(range(NCORES)), trace=trace)
            break
        except Exception:
            if attempt == 2:
                raise
            import time

            time.sleep(5)
    LAST_EXEC_NS = res.exec_time_ns
    _CACHE["last_results"] = res

    out = np.concatenate(
        [unpack_out(res.results[c]["out"], bl) for c in range(NCORES)], axis=0
    )
    return out.astype(np.float32)


# revision 17
# speedup vs baseline: 1.0336x; 1.0033x over previous
"""BiDiTreeLSTM Trainium2 kernel.

Full-input contract: kernel(**inputs) takes the unsharded numpy inputs of
reference.setup_inputs() and returns the full [64, 512] output.

Strategy: data-parallel over trees (8 trees per NeuronCore, 8 cores).
Per-core layout is feature-major: node-state tensors live in SBUF as
[128 partitions, 2 feature halves, cols], cols level-major (tree-major
within a level); children of parent column c are columns 2c/2c+1 of the
next level.

Precision plan (validated vs the jax reference in numpy sim; combined rel
err ~6e-3 against the 2e-2 gate):
 - X-projections (W_iou_bu@X, Wx_td@X) and U_iou_bu@hsum: bf16 operands
   (X plain, weights host-scaled x256).  bf16 matmul = 1 cycle/row at any
   moving dim (fp32r needs >=256), FWL halves LDWEIGHTS, X DMA halves.
 - Wh_td@h_bu, U_iou_td@h_td, U_f_{bu,td}@h: fp8e4 DoubleRow (one matmul
   per 256-contraction instead of two; 0.5 cycles/row).  Weights host-
   scaled x256; h shadows are UNSCALED plain fp8 casts (sim: scaled vs
   unscaled identical).  fp8 on U_iou_bu would fail the accuracy gate
   (2e-2) so that path stays bf16.
 - Every gate/f psum is x256; 1/256 is folded into the activation scale.
h_bu is bf16 (feeds bf16 hsum + f32-staged root output) plus an fp8 shadow
for DoubleRow consumers; TD h state is fp8-only.  fp8 DoubleRow APs keep
the feature-pair dim stride a multiple of 16 bytes.

Zero-fills exploited per the problem spec: h0 == c0 == 0, all biases == 0.
"""

import numpy as np
import ml_dtypes

B, NN, XS, H = 64, 1023, 256, 256
NCORES = 8
DEPTH = 9  # levels 0..9, level l has 2^l nodes per tree
TMAX = 512
SM_LEV = 6  # levels 0..SM_LEV-1 get batched W-projections (pre_bu / pre_td)
TOTP = 8192  # hbu8 padded col count (stride % 16 == 0)

E4 = ml_dtypes.float8_e4m3fn
BF16 = ml_dtypes.bfloat16

_CACHE = {}

LAST_EXEC_NS = None


def _levels(bl):
    levw = [bl * (1 << l) for l in range(DEPTH + 1)]
    levo = [bl * ((1 << l) - 1) for l in range(DEPTH + 1)]
    tot = bl * NN
    return levw, levo, tot


def _build_nc(bl):
    from concourse import bacc
    import concourse.mybir as mybir
    import concourse.tile as tile

    f32 = mybir.dt.float32
    bf16 = mybir.dt.bfloat16
    f8 = mybir.dt.float8e4
    DR = mybir.MatmulPerfMode.DoubleRow
    Sig = mybir.ActivationFunctionType.Sigmoid
    Tanh = mybir.ActivationFunctionType.Tanh
    DS = 1.0 / 256.0  # descale folded into the psum-reading activations

    LEVW, LEVO, TOT = _levels(bl)
    SM = LEVO[SM_LEV]  # cols of levels 0..SM_LEV-1 (contiguous, level-major)
    SMP = 512  # padded stride for the xsm tile

    nc = bacc.Bacc("TRN2", target_bir_lowering=False)

    x16_d = nc.declare_dram_parameter("x16", [256, TOT], bf16, isOutput=False)
    id_d = nc.declare_dram_parameter("ident16", [128, 128], bf16, isOutput=False)
    w16_d = nc.declare_dram_parameter("w_bu16", [256, 3 * H], bf16, isOutput=False)
    ub_d = nc.declare_dram_parameter("u_bu16", [256, 3 * H], bf16, isOutput=False)
    wx16_d = nc.declare_dram_parameter("wx_td16", [256, 3 * H], bf16, isOutput=False)
    wh8_d = nc.declare_dram_parameter("wh_td8", [256, 3 * H], f8, isOutput=False)
    ut8_d = nc.declare_dram_parameter("u_td8", [256, 3 * H], f8, isOutput=False)
    ufb8_d = nc.declare_dram_parameter("uf_bu8", [256, H], f8, isOutput=False)
    uft8_d = nc.declare_dram_parameter("uf_td8", [256, H], f8, isOutput=False)
    out_d = nc.declare_dram_parameter("out", [512, bl], f32, isOutput=True)

    with tile.TileContext(nc) as tc:
        with (
            tc.tile_pool(name="const", bufs=1) as const,
            tc.tile_pool(name="hbu_pool", bufs=1) as hbu_pool,
            tc.tile_pool(name="work", bufs=2) as work,
            tc.tile_pool(name="xtp", bufs=3) as xtp,
            tc.tile_pool(name="psg", bufs=1, space="PSUM") as psg,
            tc.tile_pool(name="psf", bufs=1, space="PSUM") as psf,
        ):
            def load_16(dram, cols, nm, eng):
                t = const.tile([128, 2, cols], bf16, name=nm, tag=nm)
                for k in (0, 1):
                    eng.dma_start(out=t[:, k, :], in_=dram[k * 128:(k + 1) * 128, :])
                return t

            def load_8(dram, cols, nm, eng):
                t = const.tile([128, 2, cols], f8, name=nm, tag=nm)
                for k in (0, 1):
                    eng.dma_start(out=t[:, k, :], in_=dram[k * 128:(k + 1) * 128, :])
                return t

            # bu weights first: w16 on the sync queue ahead of the xt
            # stream (mm#0 needs it); ub16/ufb8 on scalar (acts start later)
            w16 = load_16(w16_d, 3 * H, "w16", nc.scalar)
            ident = const.tile([128, 128], bf16, name="ident", tag="ident")
            nc.scalar.dma_start(out=ident, in_=id_d[:, :])
            ub16 = load_16(ub_d, 3 * H, "ub16", nc.scalar)
            ufb8 = load_8(ufb8_d, H, "ufb8", nc.scalar)
            # td weights are DMA'd later (emitted after the BU leaf level)
            wx16 = const.tile([128, 2, 3 * H], bf16, name="wx16", tag="wx16")
            wh8 = const.tile([128, 2, 3 * H], f8, name="wh8", tag="wh8")
            ut8 = const.tile([128, 2, 3 * H], f8, name="ut8", tag="ut8")
            uft8 = const.tile([128, 2, H], f8, name="uft8", tag="uft8")

            def load_td_weights():
                # sync queue is idle once the last BU xt tiles are in flight
                for k in (0, 1):
                    nc.sync.dma_start(out=wx16[:, k, :], in_=wx16_d[k * 128:(k + 1) * 128, :])
                    nc.sync.dma_start(out=wh8[:, k, :], in_=wh8_d[k * 128:(k + 1) * 128, :])
                    nc.sync.dma_start(out=ut8[:, k, :], in_=ut8_d[k * 128:(k + 1) * 128, :])
                    nc.sync.dma_start(out=uft8[:, k, :], in_=uft8_d[k * 128:(k + 1) * 128, :])

            # h_bu: bf16 primary (hsum, root out) + fp8 shadow (DR rhs)
            hbu = hbu_pool.tile([128, 2 * TOT], bf16, name="hbu", tag="hbu")
            hbu8 = hbu_pool.tile([128, 2, TOTP], f8, name="hbu8", tag="hbu8")
            mean = const.tile([128, 2, bl], f32, name="mean", tag="mean")
            root_f32 = const.tile([128, 2, bl], f32, name="rootf", tag="rootf")

            xsm = const.tile([128, 2, SMP], bf16, name="xsm", tag="xsm")

            # HAM-warming filler: dead matmuls on a scratch tile keep the PE
            # activity window busy through DMA startup and the tiny levels,
            # so real matmuls run at 2.4 GHz instead of the cold 1.2.
            junk = const.tile([128, 2, 512], bf16, name="junk", tag="junk")
            nc.gpsimd.memset(junk, 0.0)

            def filler(n, cols=512):
                for _ in range(n):
                    pj = psf.tile([128, cols], f32, name="junk_ps", tag="pf")
                    nc.tensor.matmul(
                        pj, junk[:, 0, 0:128], junk[:, 1, 0:cols],
                        start=True, stop=True,
                    )

            def load_xsm():
                # scalar queue: keep the sync queue clear for the leaf xt
                # stream (xsm is only needed by pre_bu, a few tiles in)
                for k in (0, 1):
                    nc.scalar.dma_start(out=xsm[:, k, 0:SM], in_=x16_d[k * 128:(k + 1) * 128, 0:SM])

            def load_x(off, o0, T):
                xt = xtp.tile([128, 2, T], bf16, name="xt", tag="xt", bufs=3)
                for k in (0, 1):
                    nc.sync.dma_start(out=xt[:, k, :], in_=x16_d[k * 128:(k + 1) * 128, off + o0:off + o0 + T])
                return xt

            def g2(ap, width):
                return ap.rearrange("p (g c) -> p g c", g=2)

            def wsl(wt, q, g):
                m = 2 * q + g
                return wt[:, :, m * 128:(m + 1) * 128]

            # ---- per-tile gate matmuls -------------------------------------
            # Three per-gate psum tiles [128, 2T] (tags pgi/pgo/pgu, bufs=1):
            # the next tile's gate-q matmuls wait only on this tile's gate-q
            # activation.  phase1 = input-ready matmuls, phase2 = recurrence-
            # dependent ones, deferred (groups left open) when each T slice
            # is a whole PSUM bank (T == 512), else merged.
            def iou_mms(T, phase1, phase2=None):
                """Small levels (T <= 256) share one [128, 4T] i|o psum tile
                (tag pgi, same banks) so a single fused Sigmoid evacuates
                both gates; big tiles keep three per-gate tiles for bank-
                level pipelining with deferred phase2."""
                fused = T <= 256
                merged = phase2 is not None and (T < 512 or fused)
                p1 = phase1 + phase2 if merged else phase1
                pending = phase2 is not None and not merged
                if fused:
                    pio = psg.tile([128, 4 * T], f32, name="pio", tag="pgi")
                    pu = psg.tile([128, 2 * T], f32, name="pu", tag="pgu")
                    pg = (pio, pu)

                    def slot(q, g):
                        if q < 2:
                            return pio[:, (2 * q + g) * T:(2 * q + g + 1) * T]
                        return pu[:, g * T:(g + 1) * T]
                else:
                    tiles = []
                    for tag in ("pgi", "pgo", "pgu"):
                        tiles.append(psg.tile([128, 2 * T], f32, name=tag, tag=tag))
                    pg = tuple(tiles)

                    def slot(q, g):
                        return pg[q][:, g * T:(g + 1) * T]

                def emit(q, g, mms, start, stop):
                    for i, (lhs, rhs, pm) in enumerate(mms):
                        nc.tensor.matmul(
                            slot(q, g), lhs, rhs,
                            start=(start and i == 0),
                            stop=(stop and i == len(mms) - 1),
                            perf_mode=pm,
                        )

                for q in range(3):
                    for g in (0, 1):
                        emit(q, g, [mk(q, g) for mk in p1], True, not pending)

                def close():
                    if not pending:
                        return
                    for q in range(3):
                        for g in (0, 1):
                            emit(q, g, [mk(q, g) for mk in phase2], False, True)

                return pg, close

            def w_makers(wt, xt, T):
                """bf16 k-pair matmul makers (contraction 256 = 2 passes)."""
                return [
                    (lambda q, g, k=k, w=wt, x=xt: (
                        w[:, k, (2 * q + g) * 128:(2 * q + g + 1) * 128],
                        x[:, k, :],
                        None,
                    ))
                    for k in (0, 1)
                ]

            def precompute_range(makers_fn, pre, lo, hi):
                """pre[:, m, lo:hi] = sum of matmuls over small cols lo:hi."""
                for m in range(6):
                    p = psf.tile([128, hi - lo], f32, name="pp", tag="pf")
                    mms = [mk(m // 2, m % 2) for mk in makers_fn(lo, hi)]
                    for i, (lhs, rhs, pm) in enumerate(mms):
                        nc.tensor.matmul(
                            p, lhs, rhs, start=(i == 0), stop=(i == len(mms) - 1),
                            perf_mode=pm,
                        )
                    nc.vector.tensor_copy(pre[:, m, lo:hi], p)

            pre_td = const.tile([128, 6, SM], bf16, name="pre_td", tag="pre2")

            def pre_maker(pre, lev):
                """identity matmul injecting pre[:, 2q+g, cols] into the
                psum group -- keeps the whole small-level chain PE->scalar."""
                return lambda q, g, p=pre, l=lev: (
                    ident,
                    p[:, 2 * q + g, LEVO[l]:LEVO[l] + LEVW[l]],
                    None,
                )

            def pre_td_makers(lo, hi):
                return [
                    (lambda q, g, k=k, a=lo, b=hi: (
                        wx16[:, k, (2 * q + g) * 128:(2 * q + g + 1) * 128],
                        xsm[:, k, a:b],
                        None,
                    ))
                    for k in (0, 1)
                ] + [
                    lambda q, g, a=lo, b=hi: (wsl(wh8, q, g), hbu8[:, :, a:b], DR)
                ]

            def gates(pg, T, c_red, c_out, h_out, h8_out=None,
                      leaf_sink=None, pre=None, lev=0):
                """pg: (pgi, pgo, pgu) big tiles or (pio, pu) fused small."""
                fused = len(pg) == 2
                if fused:
                    pio, pu = pg
                    sio = work.tile([128, 4 * T], bf16, name="sio", tag="ga", bufs=3)
                    nc.scalar.activation(sio, pio, Sig, scale=DS)
                    si, so = sio[:, 0:2 * T], sio[:, 2 * T:4 * T]
                    tu = work.tile([128, 2 * T], bf16, name="tu", tag="gb", bufs=3)
                    nc.scalar.activation(tu, pu, Tanh, scale=DS)
                else:
                    si = work.tile([128, 2 * T], bf16, name="si", tag="ga", bufs=3)
                    nc.scalar.activation(si, pg[0], Sig, scale=DS)
                    so = work.tile([128, 2 * T], bf16, name="so", tag="gb", bufs=3)
                    nc.scalar.activation(so, pg[1], Sig, scale=DS)
                    tu = work.tile([128, 2 * T], bf16, name="tu", tag="gb", bufs=3)
                    nc.scalar.activation(tu, pg[2], Tanh, scale=DS)
                if c_red is None:
                    nc.vector.tensor_mul(c_out, g2(si, T), g2(tu, T))
                else:
                    nc.vector.tensor_mul(si, si, tu)  # situ, in place
                    kind, cr = c_red
                    if kind == "full":
                        nc.vector.tensor_add(c_out, g2(si, T), cr)
                    else:  # parent-granularity c_red, broadcast to child pairs
                        pT = T // 2
                        si4 = si.rearrange("p (g n two) -> p g n two", g=2, two=2)
                        co4 = c_out.rearrange("p g (n two) -> p g n two", two=2)
                        crb = cr.to_broadcast([128, 2, pT, 2])
                        nc.vector.tensor_add(co4, si4, crb)
                tct = work.tile([128, 2 * T], bf16, name="tct", tag="ga", bufs=3)
                nc.scalar.activation(g2(tct, T), c_out, Tanh)
                if h_out is not None:
                    nc.vector.tensor_mul(h_out, g2(so, T), g2(tct, T))
                    if h8_out is not None:
                        nc.vector.tensor_copy(h8_out, h_out)
                elif h8_out is not None:
                    nc.vector.tensor_mul(h8_out, g2(so, T), g2(tct, T))
                else:
                    ht = work.tile([128, 2 * T], bf16, name="ht", tag="hsum", bufs=3)
                    nc.vector.tensor_mul(g2(ht, T), g2(so, T), g2(tct, T))
                    leaf_sink(ht)

            # ================= bottom-up =================
            filler(16, 256)
            load_xsm()
            pre_bu = const.tile([128, 6, SM], bf16, name="pre", tag="pre")
            pre_bu_makers = lambda lo, hi: [
                (lambda q, g, k=k, a=lo, b=hi: (
                    w16[:, k, (2 * q + g) * 128:(2 * q + g + 1) * 128],
                    xsm[:, k, a:b],
                    None,
                ))
                for k in (0, 1)
            ]
            with tc.tile_pool(name="bu_state", bufs=1) as bu_state:
                c_next = None
                C_next = 0
                for l in range(DEPTH, -1, -1):
                    if l == 0:
                        # pre_td for levels 1..5 overlaps the last BU levels
                        precompute_range(pre_td_makers, pre_td, LEVO[1], SM)
                    if l == SM_LEV - 1:
                        load_td_weights()
                    C, off = LEVW[l], LEVO[l]
                    T = min(TMAX, C)
                    leaf = l == DEPTH
                    small = l < SM_LEV
                    par = "A" if l % 2 else "Bp"
                    c_cur = bu_state.tile([128, 2 * C], bf16, name=f"c{l}", tag=f"c{par}")
                    choff = LEVO[l + 1] if not leaf else 0
                    ntile = C // T
                    # hsum for the whole level up front (bf16, feeds U_bu)
                    hsums = []
                    if not leaf:
                        for j in range(ntile):
                            o0 = j * T
                            hsum = work.tile([128, 2 * T], bf16, name="hsum", tag="hsum", bufs=3)
                            cb = choff + 2 * o0
                            hb4 = hbu.rearrange("p (k c) -> p k c", k=2)[
                                :, :, cb:cb + 2 * T
                            ].rearrange("p k (n two) -> p k n two", two=2)
                            nc.vector.tensor_add(
                                g2(hsum, T), hb4[:, :, :, 0], hb4[:, :, :, 1]
                            )
                            hsums.append(hsum)
                    for j in range(ntile):
                        if leaf and j == 2:
                            precompute_range(pre_bu_makers, pre_bu, 0, SM)
                        o0 = j * T
                        xt = None if small else load_x(off, o0, T)
                        cred = None
                        u_phase = None
                        pg = close = None
                        if not leaf:
                            ncj = 2 if 2 * T > TMAX else 1
                            Tc = 2 * T // ncj
                            cred = work.tile([128, 2 * T], bf16, name="cred", tag="cred")
                            hs_ = hsums[j]
                            u_phase = [
                                (lambda q, g, k=k, h=hs_, T_=T: (
                                    ub16[:, k, (2 * q + g) * 128:(2 * q + g + 1) * 128],
                                    h[:, k * T_:(k + 1) * T_],
                                    None,
                                ))
                                for k in (0, 1)
                            ]
                        if not small:
                            pg, close = iou_mms(T, w_makers(w16, xt, T), u_phase)
                        if not leaf:
                            for cj in range(ncj):
                                cb = choff + 2 * o0 + cj * Tc
                                pf = psf.tile([128, 2 * Tc], f32, name="pf", tag="pf")
                                for g in (0, 1):
                                    nc.tensor.matmul(
                                        pf[:, g * Tc:(g + 1) * Tc],
                                        wsl(ufb8, 0, g),
                                        hbu8[:, :, cb:cb + Tc],
                                        start=True, stop=True, perf_mode=DR,
                                    )
                                # f out of PSUM at the act (bank recycles now)
                                fs = work.tile([128, 2 * Tc], bf16, name="fs", tag="fs", bufs=3)
                                nc.scalar.activation(fs, pf, Sig, scale=DS)
                                cv = g2(c_next, C_next)[
                                    :, :, 2 * o0 + cj * Tc:2 * o0 + (cj + 1) * Tc
                                ]
                                fct = work.tile([128, 2 * Tc], bf16, name="fct", tag="fc")
                                nc.vector.tensor_mul(g2(fct, Tc), g2(fs, Tc), cv)
                                h2 = Tc // 2
                                crv = g2(cred, T)[:, :, cj * h2:(cj + 1) * h2]
                                fc4 = fct.rearrange("p (g n two) -> p g n two", g=2, two=2)
                                eng = nc.vector if small else nc.gpsimd
                                eng.tensor_add(crv, fc4[:, :, :, 0], fc4[:, :, :, 1])
                        if small:
                            pg, close = iou_mms(T, [pre_maker(pre_bu, l)] + u_phase)
                        else:
                            close()
                        cr = None if leaf else ("full", g2(cred, T))
                        gates(
                            pg, T, cr,
                            g2(c_cur, C)[:, :, o0:o0 + T],
                            hbu.rearrange("p (k c) -> p k c", k=2)[
                                :, :, off + o0:off + o0 + T
                            ],
                            h8_out=hbu8[:, :, off + o0:off + o0 + T],
                            lev=l,
                        )
                    c_next = c_cur
                    C_next = C

            # root h_bu staged to f32 and shipped right away
            nc.vector.tensor_copy(
                root_f32, hbu.rearrange("p (k c) -> p k c", k=2)[:, :, 0:bl]
            )
            nc.sync.dma_start(
                out=out_d[0:256, :].rearrange("(k p) b -> p k b", k=2),
                in_=root_f32,
            )

            # ---- pre_td root columns (needs the last BU output) ----
            precompute_range(pre_td_makers, pre_td, 0, LEVO[1])

            # ================= top-down =================
            with tc.tile_pool(name="td_state", bufs=1) as td_state:
                h_prev = c_prev = None
                C_prev = 0
                for l in range(0, DEPTH + 1):
                    C, off = LEVW[l], LEVO[l]
                    T = min(TMAX, C)
                    leaf = l == DEPTH
                    root = l == 0
                    small = l < SM_LEV
                    par = "A" if l % 2 else "Bp"
                    if not leaf:
                        Cp = max(C, 16)  # fp8 pair-dim stride must be %16
                        h_cur = td_state.tile([128, 2, Cp], f8, name=f"th{l}", tag=f"th{par}")
                        c_cur = td_state.tile([128, 2 * C], bf16, name=f"tc{l}", tag=f"tc{par}")
                    else:
                        h_cur = c_cur = None
                    for j in range(C // T):
                        o0 = j * T
                        xt = None if small else load_x(off, o0, T)
                        credp = None
                        pT = T // 2 if not root else 0
                        po = o0 // 2
                        u_phase = None
                        if not root:
                            hp_, po_, pT_ = h_prev, po, pT
                            u_phase = [
                                lambda q, g, h=hp_, a=po_, b=pT_: (
                                    wsl(ut8, q, g),
                                    h[:, :, a:a + b].to_broadcast([128, 2, b, 2]),
                                    DR,
                                )
                            ]
                        if not small:
                            ph1 = w_makers(wx16, xt, T) + [
                                lambda q, g, a=off + o0, T_=T: (
                                    wsl(wh8, q, g), hbu8[:, :, a:a + T_], DR)
                            ]
                            pg, close = iou_mms(T, ph1, u_phase)
                        if not root:
                            pf = psf.tile([128, 2 * pT], f32, name="pftd", tag="pf")
                            for g in (0, 1):
                                nc.tensor.matmul(
                                    pf[:, g * pT:(g + 1) * pT],
                                    wsl(uft8, 0, g),
                                    h_prev[:, :, po:po + pT],
                                    start=True, stop=True, perf_mode=DR,
                                )
                            fs = work.tile([128, 2 * pT], bf16, name="fstd", tag="fs", bufs=3)
                            nc.scalar.activation(fs, pf, Sig, scale=DS)
                            credp = work.tile([128, 2 * pT], bf16, name="credp", tag="cred")
                            nc.vector.tensor_mul(
                                g2(credp, pT), g2(fs, pT),
                                g2(c_prev, C_prev)[:, :, po:po + pT],
                            )
                        if small:
                            pg, close = iou_mms(T, [pre_maker(pre_td, l)] + (u_phase or []))
                        else:
                            close()
                        if leaf:
                            cl = work.tile([128, 2 * T], bf16, name="cl", tag="fc")
                            tree = j

                            def sink(ht, _t=tree):
                                for g in (0, 1):
                                    nc.vector.reduce_sum(
                                        mean[:, g, _t:_t + 1],
                                        ht[:, g * T:(g + 1) * T],
                                        axis=mybir.AxisListType.X,
                                    )
                                mv = mean[:, :, _t:_t + 1]
                                nc.vector.tensor_scalar_mul(
                                    mv, mv, 1.0 / (1 << DEPTH)
                                )
                                nc.sync.dma_start(
                                    out=out_d[256:512, _t:_t + 1].rearrange(
                                        "(g p) b -> p g b", g=2
                                    ),
                                    in_=mv,
                                )

                            gates(
                                pg, T, ("parent", g2(credp, pT)),
                                g2(cl, T), None, leaf_sink=sink,
                            )
                        else:
                            gates(
                                pg, T,
                                None if root else ("parent", g2(credp, pT)),
                                g2(c_cur, C)[:, :, o0:o0 + T],
                                None,
                                h8_out=h_cur[:, :, o0:o0 + T],
                            )
                    h_prev, c_prev = h_cur, c_cur
                    C_prev = C


    if not nc.is_finalized():
        nc.finalize()
    return nc


def _q8(x, scale):
    return np.clip(np.asarray(x, np.float32) * scale, -240, 240).astype(E4)


def _prep_shared(inputs):
    """Host-side weight marshaling (biases are zero by spec)."""
    f = np.ascontiguousarray
    W_iou_td = np.asarray(inputs["W_iou_td"], np.float32)
    w_bu = np.asarray(inputs["W_iou_bu"], np.float32).T  # [256, 768]
    wx_td = W_iou_td[:, :XS].T
    wh_td = f(W_iou_td[:, XS:].T)
    u_td = np.asarray(inputs["U_iou_td"], np.float32).T
    uf_bu = np.asarray(inputs["U_f_bu"], np.float32).T
    uf_td = np.asarray(inputs["U_f_td"], np.float32).T
    u_bu = np.asarray(inputs["U_iou_bu"], np.float32).T
    return {
        "ident16": np.eye(128, dtype=np.float32).astype(BF16),
        "w_bu16": f((w_bu * 256.0).astype(BF16)),
        "u_bu16": f((u_bu * 256.0).astype(BF16)),
        "wx_td16": f((wx_td * 256.0).astype(BF16)),
        "wh_td8": f(_q8(wh_td, 256.0)),
        "u_td8": f(_q8(u_td, 256.0)),
        "uf_bu8": f(_q8(uf_bu, 256.0)),
        "uf_td8": f(_q8(uf_td, 256.0)),
    }


def prep_xt(Xc):
    """[bl, NN, XS] -> level-major [256, bl*NN] bf16."""
    bl = Xc.shape[0]
    xt = np.asarray(Xc, np.float32).transpose(2, 0, 1)  # [XS, bl, NN]
    blocks = []
    for l in range(DEPTH + 1):
        lo, nl = (1 << l) - 1, 1 << l
        blocks.append(xt[:, :, lo:lo + nl].reshape(XS, bl * nl))
    return np.ascontiguousarray(np.concatenate(blocks, axis=1).astype(BF16))


def unpack_out(o, bl):
    """[512, bl] -> [bl, 512] (root_h_bu | leaf mean)."""
    return np.concatenate([o[0:256, :].T, o[256:512, :].T], axis=1)


def kernel(**inputs):
    global LAST_EXEC_NS
    from concourse.bass_utils import run_bass_kernel_spmd

    bl = B // NCORES
    if "nc" not in _CACHE:
        _CACHE["nc"] = _build_nc(bl)
    nc = _CACHE["nc"]

    shared = _prep_shared(inputs)
    X = np.asarray(inputs["X"], np.float32)
    in_maps = []
    for c in range(NCORES):
        m = dict(shared)
        m["x16"] = prep_xt(X[c * bl:(c + 1) * bl])
        in_maps.append(m)

    trace = _CACHE.get("trace", False)
    res = None
    for attempt in range(3):
        try:
            res = run_bass_kernel_spmd(nc, in_maps, list(range(NCORES)), trace=trace)
            break
        except Exception:
            if attempt == 2:
                raise
            import time

            time.sleep(5)
    LAST_EXEC_NS = res.exec_time_ns
    _CACHE["last_results"] = res

    out = np.concatenate(
        [unpack_out(res.results[c]["out"], bl) for c in range(NCORES)], axis=0
    )
    return out.astype(np.float32)
